# revision 1
# baseline (speedup 1.0000x reference)
"""Bidirectional multi-head self-attention (B=2, T=2048, C=2048, H=16, D=128,
partial RoPE over first 64 dims) on 8 TRN2 NeuronCores.

Sharding: tensor-parallel over heads. Core c computes heads (2c, 2c+1) for both
batches: qkv projection with the corresponding W_attn column slices, attention,
and the partial output  y_heads @ W_proj[head_rows, :].  The 8 partial [B,T,C]
outputs (bf16) are summed on the host (W_proj mixes heads into every output
column).

Per-core kernel layout choices:
  - x / W_attn / W_proj are fed in bf16 (PE rate is identical to fp32r, halves
    DMA bytes); intermediates (qT/kT, scores) stay fp32r.
  - x is fed transposed and chunk-blocked on the host so ONE DMA delivers a
    whole [128, 16*512] chunk (HWDGE serializes ~625ns per DMA descriptor
    chain, so DMA COUNT -- not bytes -- is the startup bottleneck).
  - RoPE: pair-swap via a 64x64 permutation matmul on TensorE + cos/sin
    elementwise combines on VectorE (cos/sin tables pre-expanded on host,
    sign folded into the sin table); 1/sqrt(D) folded into W_q on host.
  - scores are computed transposed (scT[j,i] = lhsT=kT tile, rhs=qT chunk)
    in fp32r, one [128,512] PSUM bank per (j-tile, head).
  - softmax: no max subtraction needed (scores ~ N(0,1)); exp on ScalarE
    PSUM->SBUF (bf16); denominator fused into the A@V matmul via an extra
    all-ones column appended to V.
  - A@V in bf16, interleaved with the score/exp stream per j-tile pair (two
    passes of 2 i-tiles each: PSUM zero regions are bank-granular, so only two
    accumulation groups can be live in the 2 "yu" banks); normalize on
    VectorE; transpose y via TensorE (bf16); project with W_proj slices as
    deferred PE gap-filler inside the next i-chunk's stream; outputs staged to
    [128, 2048] bf16 tiles, one DMA per 128-row block.

PSUM (8 banks = 16KB/partition): tag "qs" 2x[128,1024] (q/k accumulator pairs
in the qkv phase, score j-tile pairs in attention), tag "sm" 2x[128,512]
(v accumulators / rope shuffle / y transpose / projection accumulators), tag
"yu" 2x[128,512] A@V accumulator banks.
"""

import math
import numpy as np

from concourse import bass, bacc, mybir, tile
from concourse.bass_utils import run_bass_kernel_spmd

F32 = mybir.dt.float32
F32R = mybir.dt.float32r
BF16 = mybir.dt.bfloat16
AF = mybir.ActivationFunctionType
AO = mybir.AluOpType

N_CORES = 8
N_HEAD = 16
ROT = 64  # rotary dims per head
D = 128   # head dim
HLOC = N_HEAD // N_CORES  # heads per core = 2


def build_core_kernel(nc, tc, B, T, C):
    """Emit the per-core program. All DRAM tensors are declared on `nc` before
    the TileContext is entered; this emits into `tc`."""
    CH = 512            # t-chunk size (qkv chunks and attention i-chunks)
    NCH = T // CH
    NCT = C // 128      # contraction tiles over C
    NJT = T // 128      # j (key) tiles
    HD2 = HLOC * D      # 256
    XW = NCT * CH       # x chunk width in sbuf cols

    ap = {name: nc.tensor_map[name].ap() for name in
          ("xT", "wq", "wk", "wv", "wp", "cos_e", "sin_e", "perm", "ident", "out")}

    from contextlib import ExitStack
    ctx = ExitStack()

    wpool = ctx.enter_context(tc.tile_pool(name="wpool", bufs=1))
    xpool = ctx.enter_context(tc.tile_pool(name="xpool", bufs=3))
    qkpool = ctx.enter_context(tc.tile_pool(name="qkpool", bufs=4))
    vpool = ctx.enter_context(tc.tile_pool(name="vpool", bufs=32))
    epool = ctx.enter_context(tc.tile_pool(name="epool", bufs=12))
    ypool = ctx.enter_context(tc.tile_pool(name="ypool", bufs=9))
    spool = ctx.enter_context(tc.tile_pool(name="spool", bufs=3))
    tpool = ctx.enter_context(tc.tile_pool(name="tpool", bufs=2))
    rpool = ctx.enter_context(tc.tile_pool(name="rpool", bufs=4))
    pspool = ctx.enter_context(tc.tile_pool(name="pspool", bufs=2, space="PSUM"))

    # ---- static weights/tables + first-chunk x, interleaved for fast start ----
    wq_sb = wpool.tile([128, NCT * HD2], BF16, tag="wq")
    wk_sb = wpool.tile([128, NCT * HD2], BF16, tag="wk")
    wv_sb = wpool.tile([128, NCT * HD2], BF16, tag="wv")
    wp_sb = wpool.tile([128, HLOC * T], BF16, tag="wp")

    x_first = xpool.tile([128, XW], BF16, tag="x", name="x_b0c0")
    QW = XW // 4
    HW = NCT * HD2 // 2
    # interleave: x quarter, then a weight half, so the first q/k matmuls can
    # start after ~3 transfers instead of after the full weight set
    cos_sb = wpool.tile([ROT, T], F32, tag="cos")
    sin_sb = wpool.tile([ROT, T], F32, tag="sin")
    perm_sb = wpool.tile([ROT, ROT], F32R, tag="perm")
    ident_sb = wpool.tile([128, 128], BF16, tag="ident")

    def _pieces(total, sizes):
        o, out = 0, []
        for s in sizes:
            out.append((o, o + s))
            o += s
        assert o == total
        return out

    # startup order tuned for the serial ~625ns/DMA HWDGE setup chain: the
    # first q/k matmuls need only (x 2-ct piece, wq 4-ct piece, wk 4-ct piece)
    xp = _pieces(XW, [2 * CH, 2 * CH, 4 * CH, 4 * CH, 4 * CH])
    wqp = _pieces(NCT * HD2, [4 * HD2, 12 * HD2])
    wvp = _pieces(NCT * HD2, [8 * HD2, 8 * HD2])
    seqs = [
        ("x", xp[0]), ("wq", wqp[0]), ("wk", wqp[0]), ("x", xp[1]),
        ("perm", None), ("wq", wqp[1]), ("wk", wqp[1]), ("x", xp[2]),
        ("cos", None), ("wv", wvp[0]), ("x", xp[3]), ("sin", None),
        ("wv", wvp[1]), ("x", xp[4]), ("ident", None),
    ]
    for kind, pc in seqs:
        if kind == "x":
            nc.sync.dma_start(x_first[:, pc[0]:pc[1]], ap["xT"][0, 0, :, pc[0]:pc[1]])
        elif kind in ("wq", "wk", "wv"):
            sb = {"wq": wq_sb, "wk": wk_sb, "wv": wv_sb}[kind]
            nc.sync.dma_start(sb[:, pc[0]:pc[1]], ap[kind][:, pc[0]:pc[1]])
        elif kind == "perm":
            nc.sync.dma_start(perm_sb[:, :], ap["perm"][:, :])
        elif kind == "cos":
            nc.sync.dma_start(cos_sb[:, :], ap["cos_e"][:, :])
        elif kind == "sin":
            nc.sync.dma_start(sin_sb[:, :], ap["sin_e"][:, :])
        elif kind == "ident":
            nc.sync.dma_start(ident_sb[:, :], ap["ident"][:, :])
    zbias = wpool.tile([128, 1], F32, tag="zbias")
    nc.gpsimd.memset(zbias[:, :], 0.0)

    x_pre = {(0, 0): x_first}

    def fetch_x(b, ch):
        if (b, ch) in x_pre:
            return x_pre.pop((b, ch))
        xt = xpool.tile([128, XW], BF16, tag="x", name=f"x_b{b}c{ch}")
        nc.sync.dma_start(xt[:, :], ap["xT"][b, ch, :, :])
        return xt

    for b in range(B):
        # ================= QKV projection + RoPE =================
        qT = {}
        kT = {}
        for h in range(HLOC):
            qT[h] = qkpool.tile([128, T], F32R, tag="qkT", name=f"qT_b{b}h{h}")
            kT[h] = qkpool.tile([128, T], F32R, tag="qkT", name=f"kT_b{b}h{h}")
        vaug = {}

        pre_exp = {}
        for ch in range(NCH):
            tsl = slice(ch * CH, (ch + 1) * CH)
            xt = fetch_x(b, ch)

            # --- q,k accumulation: both heads packed per [128,1024] psum pair ---
            qaccp = pspool.tile([128, 2 * CH], F32, tag="qs", bufs=2,
                                name=f"qacc_b{b}c{ch}")
            kaccp = pspool.tile([128, 2 * CH], F32, tag="qs", bufs=2,
                                name=f"kacc_b{b}c{ch}")
            for ct in range(NCT):
                xsl = xt[:, ct * CH:(ct + 1) * CH]
                for h in range(HLOC):
                    nc.tensor.matmul(
                        qaccp[:, h * CH:(h + 1) * CH],
                        wq_sb[:, ct * HD2 + h * D: ct * HD2 + (h + 1) * D],
                        xsl, start=(ct == 0), stop=(ct == NCT - 1))
                    nc.tensor.matmul(
                        kaccp[:, h * CH:(h + 1) * CH],
                        wk_sb[:, ct * HD2 + h * D: ct * HD2 + (h + 1) * D],
                        xsl, start=(ct == 0), stop=(ct == NCT - 1))

            # copy q/k out of PSUM. k first (scores need every kT tile before
            # the attention phase), except on the last chunk where the q PSUM
            # pair is what the first score tile's ring slot waits on. On chunk
            # 0 the copies go AFTER the v-loop emission so DVE prioritizes the
            # va copies the v accumulator ring is waiting on.
            order = ((kaccp, kT), (qaccp, qT))
            if ch == NCH - 1:
                order = ((qaccp, qT), (kaccp, kT))

            def emit_qk_copies():
                for (acc, tgts) in order:
                    for h in range(HLOC):
                        nc.vector.tensor_copy(tgts[h][:, tsl], acc[:, h * CH:(h + 1) * CH])

            if ch != 0:
                emit_qk_copies()

            # --- v accumulation (overlaps the rope copies on DVE); on the last
            # chunk, the first attention chunk's first score pairs + exps
            # (head 0, j-tiles 0..7: their kT/qT chunks are already roped) are
            # interleaved between v iterations so ScalarE gets a ~4-pair head
            # start on the attention phase instead of trailing it ---
            for tt in range(CH // 128):
                vacc = pspool.tile([128, HD2], F32, tag="sm", bufs=2,
                                   name=f"vacc_b{b}c{ch}t{tt}")
                for ct in range(NCT):
                    nc.tensor.matmul(
                        vacc[:, :],
                        xt[:, ct * CH + tt * 128: ct * CH + (tt + 1) * 128],
                        wv_sb[:, ct * HD2:(ct + 1) * HD2],
                        start=(ct == 0), stop=(ct == NCT - 1))
                jt = ch * (CH // 128) + tt
                for h in range(HLOC):
                    va = vpool.tile([128, 130], BF16, tag="vaug", name=f"va_b{b}h{h}j{jt}")
                    nc.vector.tensor_copy(va[:, 0:D], vacc[:, h * D:(h + 1) * D])
                    nc.gpsimd.memset(va[:, D:D + 1], 1.0)
                    vaug[(h, jt)] = va
                if ch == NCH - 1:
                    jp = tt
                    scp = pspool.tile([128, 2 * CH], F32, tag="qs", bufs=2,
                                      name=f"scp_b{b}i0h0p{jp}")
                    for k in range(2):
                        jjt = 2 * jp + k
                        nc.tensor.matmul(scp[:, k * CH:(k + 1) * CH],
                                         kT[0][:, jjt * 128:(jjt + 1) * 128],
                                         qT[0][:, 0:CH],
                                         start=True, stop=True)
                    ep = epool.tile([128, 2 * CH], BF16, tag="expT",
                                    name=f"e_b{b}i0h0p{jp}")
                    nc.scalar.activation(ep[:, :], scp[:, :], AF.Exp, bias=zbias[:, 0:1])
                    pre_exp[jp] = ep

            if ch == 0:
                emit_qk_copies()

            # --- RoPE rotate (k first) ---
            for tgts in (kT, qT):
                for h in range(HLOC):
                    tgt = tgts[h]
                    shuf = pspool.tile([ROT, CH], F32, tag="sm", bufs=2,
                                       name=f"shuf_b{b}c{ch}h{h}")
                    nc.tensor.matmul(shuf[:, :], perm_sb[:, :], tgt[0:ROT, tsl],
                                     start=True, stop=True)
                    tmp1 = tpool.tile([ROT, CH], F32, tag="tmp", name="rtmp1")
                    tmp2 = tpool.tile([ROT, CH], F32, tag="tmp", name="rtmp2")
                    nc.vector.tensor_mul(tmp1[:, :], shuf[:, :], sin_sb[:, tsl])
                    nc.vector.tensor_mul(tmp2[:, :], tgt[0:ROT, tsl], cos_sb[:, tsl])
                    nc.vector.tensor_add(tgt[0:ROT, tsl], tmp1[:, :], tmp2[:, :])

            if b == 0 and ch == min(1, NCH - 1):
                nc.sync.dma_start(wp_sb[:, :], ap["wp"][:, :])

        # ================= attention + projection =================
        proj_prev = None  # (ich, yTs) of the previous i-chunk, projected
        #                    lazily inside the next i-chunk's PE stream

        def emit_proj_task(task):
            """One projection column block: 2 accumulating matmuls + staging
            copy; fires the row-block DMA when its last column lands."""
            pich, pit, cc, yTs_p, st = task
            row0 = pich * CH + pit * 128
            pacc = pspool.tile([128, CH], F32, tag="sm", bufs=2,
                               name=f"pacc_b{b}i{pich}t{pit}c{cc}")
            for hh in range(HLOC):
                nc.tensor.matmul(pacc[:, :],
                                 yTs_p[(hh, pit)][:, :],
                                 wp_sb[:, hh * T + cc * CH:hh * T + (cc + 1) * CH],
                                 start=(hh == 0), stop=(hh == HLOC - 1))
            nc.vector.tensor_copy(st[:, cc * CH:(cc + 1) * CH], pacc[:, :])
            if cc == C // CH - 1:
                nc.sync.dma_start(ap["out"][b, row0:row0 + 128, :], st[:, :])

        def proj_tasks_of(pich, yTs_p):
            tasks = []
            for pit in range(CH // 128):
                st = spool.tile([128, C], BF16, tag="ostage",
                                name=f"st_b{b}i{pich}t{pit}")
                for cc in range(C // CH):
                    tasks.append((pich, pit, cc, yTs_p, st))
            return tasks

        norm_q = []   # A@V accumulators awaiting normalize (rec+yn on DVE)
        normb_q = []  # normalized yn tiles awaiting transpose (PE) + copy

        def run_norm_a(entry):
            """Stage a: reciprocal + scale on DVE — this is what releases the
            A@V PSUM banks for the next head."""
            tgt, pich, h, it, yus = entry
            rec = rpool.tile([128, 1], F32, tag="rec", name="rec")
            nc.vector.reciprocal(rec[:, :], yus[:, D:D + 1])
            yn = tpool.tile([128, 128], BF16, tag="yn", bufs=8,
                            name=f"yn_b{b}h{h}i{pich}t{it}")
            nc.vector.tensor_scalar_mul(yn[:, :], yus[:, 0:D], rec[:, 0:1])
            normb_q.append((tgt, pich, h, it, yn))

        def run_norm_b(entry):
            """Stage b: PE transpose + staging copy, two j-pairs later so the
            DVE chain from stage a never stalls the PE."""
            tgt, pich, h, it, yn = entry
            tp = pspool.tile([128, 128], BF16, tag="sm", bufs=2,
                             name=f"tp_b{b}h{h}i{pich}t{it}")
            nc.tensor.transpose(tp[:, :], yn[:, :], ident_sb[:, :])
            yt = ypool.tile([128, 128], BF16, tag="yT", name=f"yt_b{b}h{h}i{pich}t{it}")
            nc.vector.tensor_copy(yt[:, :], tp[:, :])
            tgt[(h, it)] = yt

        for ich in range(NCH):
            isl = slice(ich * CH, (ich + 1) * CH)
            # prefetch next batch's first x chunks during this attention phase
            if b == 0 and ich in (0, 1) and B > 1:
                xt = xpool.tile([128, XW], BF16, tag="x", name=f"x_b1c{ich}")
                nc.sync.dma_start(xt[:, :], ap["xT"][1, ich, :, :])
                x_pre[(1, ich)] = xt

            pending = proj_tasks_of(*proj_prev) if proj_prev is not None else []
            pending.reverse()  # pop() from the front in order
            yTs = {}
            NIT = CH // 128
            yu_of = {}

            NPJ = NJT // 2  # j-tile pairs
            PLAG = 4        # pairs between a score pair and its A@V consumers
            for h in range(HLOC):
                # scores and A@V interleaved per j-tile pair: two score matmuls
                # fill halves of one [128,1024] PSUM tile, ONE exp covers both
                # (the ScalarE per-instruction init is the attention phase's
                # co-critical cost). PSUM zero regions are bank-granular, so
                # only 2 A@V accumulation groups can be live (one per "yu"
                # bank): pass 1 accumulates i-tiles 0,1 alongside the score
                # stream, pass 2 accumulates i-tiles 2,3 from the SBUF exp
                # tiles afterwards.
                expp = {}
                for it in (0, 1):
                    yu = pspool.tile([128, 512], F32, tag="yu", bufs=2,
                                     name=f"yu_b{b}h{h}i{ich}t{it}")
                    yu_of[(h, it)] = yu

                for jp in range(NPJ + PLAG):
                    if jp < NPJ:
                        if ich == 0 and h == 0 and jp in pre_exp:
                            expp[jp] = pre_exp.pop(jp)
                        else:
                            scp = pspool.tile([128, 2 * CH], F32, tag="qs", bufs=2,
                                              name=f"scp_b{b}i{ich}h{h}p{jp}")
                            for k in range(2):
                                jt = 2 * jp + k
                                nc.tensor.matmul(scp[:, k * CH:(k + 1) * CH],
                                                 kT[h][:, jt * 128:(jt + 1) * 128],
                                                 qT[h][:, isl],
                                                 start=True, stop=True)
                            ep = epool.tile([128, 2 * CH], BF16, tag="expT",
                                            name=f"e_b{b}i{ich}h{h}p{jp}")
                            nc.scalar.activation(ep[:, :], scp[:, :], AF.Exp, bias=zbias[:, 0:1])
                            expp[jp] = ep
                    # drain deferred normalizes (the previous head's pass-2
                    # accumulators): stage a at jp=0,1 frees those banks for
                    # this head's pass-1 writes at jp=PLAG; stage b follows
                    # when the DVE chain has certainly finished
                    if jp in (0, 1):
                        if norm_q:
                            run_norm_a(norm_q.pop(0))
                    if jp in (1, 2):
                        if normb_q:
                            run_norm_b(normb_q.pop(0))
                    # previous i-chunk's projection as PE gap-filler: keeps the
                    # PE ahead of ScalarE's exp stream instead of stalling on it
                    if pending and 2 <= jp:
                        emit_proj_task(pending.pop())
                    if jp >= PLAG:
                        p = jp - PLAG
                        for k in range(2):
                            j = 2 * p + k
                            for it in (0, 1):
                                nc.tensor.matmul(yu_of[(h, it)][:, 0:D + 1],
                                                 expp[p][:, k * CH + it * 128:k * CH + (it + 1) * 128],
                                                 vaug[(h, j)][:, 0:D + 1],
                                                 start=(j == 0), stop=(j == NJT - 1))
                # pass 2: i-tiles 2,3 accumulate from the SBUF exp tiles while
                # the normalize/projection fillers keep the other engines fed
                for it in (2, 3):
                    yu = pspool.tile([128, 512], F32, tag="yu", bufs=2,
                                     name=f"yu_b{b}h{h}i{ich}t{it}")
                    yu_of[(h, it)] = yu
                for q in range(NPJ):
                    if q == 0:
                        run_norm_a((yTs, ich, h, 0, yu_of[(h, 0)]))
                        run_norm_a((yTs, ich, h, 1, yu_of[(h, 1)]))
                    if q in (2, 3):
                        run_norm_b(normb_q.pop(0))
                    for k in range(2):
                        j = 2 * q + k
                        for it in (2, 3):
                            nc.tensor.matmul(yu_of[(h, it)][:, 0:D + 1],
                                             expp[q][:, k * CH + it * 128:k * CH + (it + 1) * 128],
                                             vaug[(h, j)][:, 0:D + 1],
                                             start=(j == 0), stop=(j == NJT - 1))
                # pass-2 normalizes run inside the next head's (or chunk's) bursts
                for it in (2, 3):
                    norm_q.append((yTs, ich, h, it, yu_of[(h, it)]))
            while pending:
                emit_proj_task(pending.pop())
            proj_prev = (ich, yTs)

        # trailing work for this batch's last i-chunk: interleave the
        # normalize chain one step ahead of the projection so the PE keeps
        # streaming
        run_norm_a(norm_q.pop(0))
        run_norm_a(norm_q.pop(0))
        run_norm_b(normb_q.pop(0))
        tail_tasks = proj_tasks_of(*proj_prev)
        for it in range(CH // 128):
            if norm_q:
                run_norm_a(norm_q.pop(0))
            if normb_q:
                run_norm_b(normb_q.pop(0))
            for task in tail_tasks[4 * it:4 * it + 4]:
                emit_proj_task(task)

    ctx.close()


def make_nc(B=2, T=2048, C=2048, reps=1, loop=1):
    nc = bacc.Bacc("TRN2", target_bir_lowering=False, debug=False)
    nc.tensor_map = {}

    def dram(name, shape, kind, dt=F32):
        t = nc.dram_tensor(name, shape, dt, kind=kind)
        nc.tensor_map[name] = t
        return t

    CH = 512
    NCH = T // CH
    NCT = C // 128
    dram("xT", [B, NCH, 128, NCT * CH], "ExternalInput", BF16)
    dram("wq", [128, NCT * HLOC * D], "ExternalInput", BF16)
    dram("wk", [128, NCT * HLOC * D], "ExternalInput", BF16)
    dram("wv", [128, NCT * HLOC * D], "ExternalInput", BF16)
    dram("wp", [128, HLOC * T], "ExternalInput", BF16)
    dram("cos_e", [ROT, T], "ExternalInput")
    dram("sin_e", [ROT, T], "ExternalInput")
    dram("perm", [ROT, ROT], "ExternalInput", F32R)
    dram("ident", [128, 128], "ExternalInput", BF16)
    dram("out", [B, T, C], "ExternalOutput", BF16)

    with tile.TileContext(nc) as tc:
        if loop > 1:
            with tc.For_i(0, loop, 1):
                build_core_kernel(nc, tc, B, T, C)
        else:
            for _ in range(reps):
                build_core_kernel(nc, tc, B, T, C)
    nc.compile()
    return nc


def host_inputs(x, W_attn, W_proj, cos, sin):
    """Build the 8 per-core input maps from the full-problem inputs."""
    import ml_dtypes
    BF = ml_dtypes.bfloat16
    B, T, C = x.shape
    CH = 512
    NCH = T // CH
    NCT = C // 128
    # xr[b, ch, p, ct*CH + c] = x[b, ch*CH + c, ct*128 + p]
    xr = np.ascontiguousarray(
        x.reshape(B, NCH, CH, NCT, 128).transpose(0, 1, 4, 3, 2)
        .reshape(B, NCH, 128, NCT * CH)).astype(BF)
    cos_e = np.repeat(np.ascontiguousarray(cos.T), 2, axis=0).astype(np.float32)
    sin_e = np.repeat(np.ascontiguousarray(sin.T), 2, axis=0).astype(np.float32)
    sin_e[0::2, :] *= -1.0
    perm = np.zeros((ROT, ROT), np.float32)
    for i in range(ROT):
        perm[i, i ^ 1] = 1.0
    ident = np.eye(128, dtype=np.float32).astype(BF)
    scale = 1.0 / math.sqrt(D)

    def wlayout(w):
        # [C, HLOC*D] -> [128, NCT * HLOC*D] with ct-major column blocks
        return np.ascontiguousarray(
            w.reshape(NCT, 128, HLOC * D).transpose(1, 0, 2).reshape(128, -1)).astype(BF)

    in_maps = []
    for core in range(N_CORES):
        h0 = core * HLOC
        sl = slice(h0 * D, (h0 + HLOC) * D)
        wp = np.ascontiguousarray(
            W_proj[sl, :].reshape(HLOC, 128, C).transpose(1, 0, 2).reshape(128, -1)).astype(BF)
        in_maps.append({
            "xT": xr,
            "wq": wlayout(W_attn[:, sl] * scale),
            "wk": wlayout(W_attn[:, C + h0 * D: C + (h0 + HLOC) * D]),
            "wv": wlayout(W_attn[:, 2 * C + h0 * D: 2 * C + (h0 + HLOC) * D]),
            "wp": wp,
            "cos_e": cos_e,
            "sin_e": sin_e,
            "perm": perm,
            "ident": ident,
        })
    return in_maps


_NC_CACHE = {}


def build_runner(nc):
    """Build a sharded jitted runner over 8 cores for an arbitrary nc, modeled
    on concourse.bass2jax.run_bass_via_pjrt but with a cached executable."""
    import jax
    from jax.sharding import Mesh, PartitionSpec
    from jax.experimental.shard_map import shard_map
    from concourse import bass2jax

    bass2jax.install_neuronx_cc_hook()

    partition_name = nc.partition_id_tensor.name if nc.partition_id_tensor else None
    in_names, out_names, out_avals, zero_shapes = [], [], [], []
    for alloc in nc.m.functions[0].allocations:
        if not isinstance(alloc, mybir.MemoryLocationSet):
            continue
        name = alloc.memorylocations[0].name
        if alloc.kind == "ExternalInput":
            if name != partition_name:
                in_names.append(name)
        elif alloc.kind == "ExternalOutput":
            out_names.append(name)
            shape = tuple(alloc.tensor_shape)
            dtype = mybir.dt.np(alloc.dtype)
            out_avals.append(jax.core.ShapedArray(shape, dtype))
            zero_shapes.append((shape, dtype))
    n_params = len(in_names)
    n_outs = len(out_names)
    all_names = in_names + out_names
    if partition_name is not None:
        all_names = all_names + [partition_name]

    def _body(*args):
        operands = list(args)
        if partition_name is not None:
            operands.append(bass2jax.partition_id_tensor())
        outs = bass2jax._bass_exec_p.bind(
            *operands,
            out_avals=tuple(out_avals),
            in_names=tuple(all_names),
            out_names=tuple(out_names),
            lowering_input_output_aliases=(),
            sim_require_finite=True,
            sim_require_nnan=True,
            nc=nc,
        )
        return tuple(outs)

    devices = jax.devices()[:N_CORES]
    mesh = Mesh(np.asarray(devices), ("core",))
    in_specs = (PartitionSpec("core"),) * (n_params + n_outs)
    out_specs = (PartitionSpec("core"),) * n_outs
    donate = tuple(range(n_params, n_params + n_outs))
    sharded = jax.jit(
        shard_map(_body, mesh=mesh, in_specs=in_specs, out_specs=out_specs,
                  check_rep=False),
        donate_argnums=donate, keep_unused=True)

    runner = {
        "fn": sharded, "in_names": in_names, "out_names": out_names,
        "zero_shapes": zero_shapes, "n_params": n_params, "mesh": mesh,
    }
    return runner


def _get_runner(reps=1, loop=1):
    key = ("runner", reps, loop)
    if key not in _NC_CACHE:
        _NC_CACHE[key] = build_runner(make_nc(reps=reps, loop=loop))
    return _NC_CACHE[key]


def _concat_inputs(runner, in_maps):
    return [np.concatenate([in_maps[c][name] for c in range(N_CORES)], axis=0)
            for name in runner["in_names"]]


def _make_zeros(runner):
    return [np.zeros((N_CORES * s[0], *s[1:]), dt)
            for (s, dt) in runner["zero_shapes"]]


def kernel(x, W_attn, W_proj, cos, sin):
    x = np.asarray(x, np.float32)
    W_attn = np.asarray(W_attn, np.float32)
    W_proj = np.asarray(W_proj, np.float32)
    cos = np.asarray(cos, np.float32)
    sin = np.asarray(sin, np.float32)

    runner = _get_runner()
    in_maps = host_inputs(x, W_attn, W_proj, cos, sin)
    out_arrs = runner["fn"](*_concat_inputs(runner, in_maps), *_make_zeros(runner))
    B, T, C = x.shape
    parts = np.asarray(out_arrs[0]).astype(np.float32).reshape(N_CORES, B, T, C)
    return parts.sum(axis=0, dtype=np.float32)


def bench(x, W_attn, W_proj, cos, sin, iters=10, reps=1, loop=1):
    """Time device-resident executions; returns (output, per-iter seconds list)."""
    import time
    import jax
    from jax.sharding import NamedSharding, PartitionSpec

    runner = _get_runner(reps=reps, loop=loop)
    in_maps = host_inputs(np.asarray(x, np.float32), np.asarray(W_attn, np.float32),
                          np.asarray(W_proj, np.float32), np.asarray(cos, np.float32),
                          np.asarray(sin, np.float32))
    sharding = NamedSharding(runner["mesh"], PartitionSpec("core"))
    dev_in = [jax.device_put(a, sharding) for a in _concat_inputs(runner, in_maps)]
    zero_sets = [[jax.device_put(z, sharding) for z in _make_zeros(runner)]
                 for _ in range(iters + 1)]
    for z in zero_sets:
        jax.block_until_ready(z)
    jax.block_until_ready(dev_in)

    # warmup (also compiles)
    out = runner["fn"](*dev_in, *zero_sets[0])
    jax.block_until_ready(out)
    times = []
    for i in range(iters):
        t0 = time.perf_counter()
        out = runner["fn"](*dev_in, *zero_sets[i + 1])
        jax.block_until_ready(out)
        times.append(time.perf_counter() - t0)
    B, T, C = np.asarray(x).shape
    parts = np.asarray(out[0]).astype(np.float32).reshape(N_CORES, B, T, C)
    return parts.sum(axis=0, dtype=np.float32), times


if __name__ == "__main__":
    np.random.seed(0)
    B, T, C = 2, 2048, 2048
    x = np.random.randn(B, T, C).astype(np.float32)
    W_attn = (np.random.randn(C, 3 * C) / math.sqrt(C)).astype(np.float32)
    W_proj = (np.random.randn(C, C) / math.sqrt(C)).astype(np.float32)
    half = ROT // 2
    inv = 1.0 / 10000.0 ** (np.arange(half, dtype=np.float32) / half)
    ang = np.outer(np.arange(T, dtype=np.float32), inv)
    out = kernel(x, W_attn, W_proj, np.cos(ang), np.sin(ang))
    print(out.shape, out.dtype)



# revision 17
# speedup vs baseline: 2.1254x; 2.1254x over previous
"""Bidirectional multi-head self-attention (B=2, T=2048, C=2048, H=16, D=128,
partial RoPE over first 64 dims) on 8 TRN2 NeuronCores.

Sharding: tensor-parallel over heads. Core c computes heads (2c, 2c+1) for both
batches: qkv projection with the corresponding W_attn column slices, attention,
and the partial output  y_heads @ W_proj[head_rows, :].  The 8 partial [B,T,C]
outputs (bf16) are summed on the host (W_proj mixes heads into every output
column).

Per-core kernel layout choices:
  - x / W_attn / W_proj are fed in bf16 (PE rate is identical to fp32r, halves
    DMA bytes); intermediates (qT/kT, scores) stay fp32r.
  - x is fed transposed and chunk-blocked on the host so ONE DMA delivers a
    whole [128, 16*512] chunk (HWDGE serializes ~625ns per DMA descriptor
    chain, so DMA COUNT -- not bytes -- is the startup bottleneck).
  - RoPE: pair-swap via a 64x64 permutation matmul on TensorE + cos/sin
    elementwise combines on VectorE (cos/sin tables pre-expanded on host,
    sign folded into the sin table); 1/sqrt(D) folded into W_q on host.
  - scores are computed transposed (scT[j,i] = lhsT=kT tile, rhs=qT chunk)
    in fp32r, one [128,512] PSUM bank per (j-tile, head).
  - softmax: no max subtraction needed (scores ~ N(0,1)); exp on ScalarE
    PSUM->SBUF (bf16); denominator fused into the A@V matmul via an extra
    all-ones column appended to V.
  - A@V in bf16, interleaved with the score/exp stream per j-tile pair (two
    passes of 2 i-tiles each: PSUM zero regions are bank-granular, so only two
    accumulation groups can be live in the 2 "yu" banks); normalize on
    VectorE; transpose y via TensorE (bf16); project with W_proj slices as
    deferred PE gap-filler inside the next i-chunk's stream; outputs staged to
    [128, 2048] bf16 tiles, one DMA per 128-row block.

PSUM (8 banks = 16KB/partition): tag "qs" 2x[128,1024] (q/k accumulator pairs
in the qkv phase, score j-tile pairs in attention), tag "sm" 2x[128,512]
(v accumulators / rope shuffle / y transpose / projection accumulators), tag
"yu" 2x[128,512] A@V accumulator banks.
"""

import math
import numpy as np

from concourse import bass, bacc, mybir, tile
from concourse.bass_utils import run_bass_kernel_spmd

F32 = mybir.dt.float32
F32R = mybir.dt.float32r
BF16 = mybir.dt.bfloat16
AF = mybir.ActivationFunctionType
AO = mybir.AluOpType

N_CORES = 8
N_HEAD = 16
ROT = 64  # rotary dims per head
D = 128   # head dim
HLOC = N_HEAD // N_CORES  # heads per core = 2


def build_core_kernel(nc, tc, B, T, C):
    """Emit the per-core program. All DRAM tensors are declared on `nc` before
    the TileContext is entered; this emits into `tc`."""
    CH = 512            # t-chunk size (qkv chunks and attention i-chunks)
    NCH = T // CH
    NCT = C // 128      # contraction tiles over C
    NJT = T // 128      # j (key) tiles
    HD2 = HLOC * D      # 256
    XW = NCT * CH       # x chunk width in sbuf cols

    ap = {name: nc.tensor_map[name].ap() for name in
          ("xT", "wq", "wk", "wv", "wp", "cos_e", "sin_e", "perm", "ident", "out")}

    from contextlib import ExitStack
    ctx = ExitStack()

    wpool = ctx.enter_context(tc.tile_pool(name="wpool", bufs=1))
    xpool = ctx.enter_context(tc.tile_pool(name="xpool", bufs=3))
    qkpool = ctx.enter_context(tc.tile_pool(name="qkpool", bufs=4))
    vpool = ctx.enter_context(tc.tile_pool(name="vpool", bufs=32))
    epool = ctx.enter_context(tc.tile_pool(name="epool", bufs=12))
    ypool = ctx.enter_context(tc.tile_pool(name="ypool", bufs=9))
    spool = ctx.enter_context(tc.tile_pool(name="spool", bufs=3))
    tpool = ctx.enter_context(tc.tile_pool(name="tpool", bufs=2))
    rpool = ctx.enter_context(tc.tile_pool(name="rpool", bufs=4))
    pspool = ctx.enter_context(tc.tile_pool(name="pspool", bufs=2, space="PSUM"))

    # ---- static weights/tables + first-chunk x, interleaved for fast start ----
    wq_sb = wpool.tile([128, NCT * HD2], BF16, tag="wq")
    wk_sb = wpool.tile([128, NCT * HD2], BF16, tag="wk")
    wv_sb = wpool.tile([128, NCT * HD2], BF16, tag="wv")
    wp_sb = wpool.tile([128, HLOC * T], BF16, tag="wp")

    x_first = xpool.tile([128, XW], BF16, tag="x", name="x_b0c0")
    QW = XW // 4
    HW = NCT * HD2 // 2
    # interleave: x quarter, then a weight half, so the first q/k matmuls can
    # start after ~3 transfers instead of after the full weight set
    cos_sb = wpool.tile([ROT, T], F32, tag="cos")
    sin_sb = wpool.tile([ROT, T], F32, tag="sin")
    perm_sb = wpool.tile([ROT, ROT], F32R, tag="perm")
    ident_sb = wpool.tile([128, 128], BF16, tag="ident")

    def _pieces(total, sizes):
        o, out = 0, []
        for s in sizes:
            out.append((o, o + s))
            o += s
        assert o == total
        return out

    # startup order tuned for the serial ~625ns/DMA HWDGE setup chain: the
    # first q/k matmuls need only (x 1-ct piece, wq 1-ct piece, wk 1-ct
    # piece); x(0,1) is prefetched at the end of the chain (needed ~20us in),
    # ident (first used in the attention phase ~90us in) goes last
    xp = _pieces(XW, [2 * CH, 2 * CH, 4 * CH, 4 * CH, 4 * CH])
    wqp = _pieces(NCT * HD2, [4 * HD2, 12 * HD2])
    wvp = _pieces(NCT * HD2, [8 * HD2, 8 * HD2])
    x01p = _pieces(XW, [8 * CH, 8 * CH])
    x_next = xpool.tile([128, XW], BF16, tag="x", name="x_b0c1")
    seqs = [
        ("x", xp[0]), ("wq", wqp[0]), ("wk", wqp[0]), ("x", xp[1]),
        ("perm", None), ("wq", wqp[1]), ("wk", wqp[1]), ("x", xp[2]),
        ("cos", None), ("wv", wvp[0]), ("x", xp[3]), ("sin", None),
        ("wv", wvp[1]), ("x", xp[4]),
        ("x01", x01p[0]), ("x01", x01p[1]), ("ident", None),
    ]
    for kind, pc in seqs:
        if kind == "x":
            nc.sync.dma_start(x_first[:, pc[0]:pc[1]], ap["xT"][0, 0, :, pc[0]:pc[1]])
        elif kind == "x01":
            nc.sync.dma_start(x_next[:, pc[0]:pc[1]], ap["xT"][0, 1, :, pc[0]:pc[1]])
        elif kind in ("wq", "wk", "wv"):
            sb = {"wq": wq_sb, "wk": wk_sb, "wv": wv_sb}[kind]
            nc.sync.dma_start(sb[:, pc[0]:pc[1]], ap[kind][:, pc[0]:pc[1]])
        elif kind == "perm":
            nc.sync.dma_start(perm_sb[:, :], ap["perm"][:, :])
        elif kind == "cos":
            nc.sync.dma_start(cos_sb[:, :], ap["cos_e"][:, :])
        elif kind == "sin":
            nc.sync.dma_start(sin_sb[:, :], ap["sin_e"][:, :])
        elif kind == "ident":
            nc.sync.dma_start(ident_sb[:, :], ap["ident"][:, :])
    zbias = wpool.tile([128, 1], F32, tag="zbias")
    nc.gpsimd.memset(zbias[:, :], 0.0)

    x_pre = {(0, 0): x_first, (0, 1): x_next}

    def fetch_x(b, ch):
        if (b, ch) in x_pre:
            return x_pre.pop((b, ch))
        xt = xpool.tile([128, XW], BF16, tag="x", name=f"x_b{b}c{ch}")
        nc.sync.dma_start(xt[:, :], ap["xT"][b, ch, :, :])
        return xt

    # Projection/normalize pipeline state carried ACROSS i-chunks and across
    # the batch boundary: batch b's first i-chunk projects batch b-1's last
    # i-chunk as PE gap-filler (the baseline drained it in a serial tail).
    proj_prev = None  # (pb, pich, yTs) of the previous i-chunk
    norm_q = []   # A@V accumulators awaiting normalize (rec+yn on DVE)
    normb_q = []  # normalized yn tiles awaiting transpose (PE) + copy

    def emit_proj_task(task, tail=False, final=False):
        """One projection column block: 2 accumulating matmuls + staging
        copy; fires the row-block DMA when its last column lands.  In the
        end-of-kernel tail the staging copy alternates DVE/Act (the tail is
        DVE-queue-bound) and the last row block DMAs per column block to
        shorten the drain."""
        pb, pich, pit, cc, yTs_p, st = task
        row0 = pich * CH + pit * 128
        ptag = "yu" if (tail and pit >= 2 and cc % 2 == 1) else "sm"
        pacc = pspool.tile([128, CH], F32, tag=ptag, bufs=2,
                           name=f"pacc_b{pb}i{pich}t{pit}c{cc}")
        for hh in range(HLOC):
            nc.tensor.matmul(pacc[:, :],
                             yTs_p[(hh, pit)][:, :],
                             wp_sb[:, hh * T + cc * CH:hh * T + (cc + 1) * CH],
                             start=(hh == 0), stop=(hh == HLOC - 1))
        if tail and cc % 2 == 1:
            nc.scalar.copy(st[:, cc * CH:(cc + 1) * CH], pacc[:, :])
        else:
            nc.vector.tensor_copy(st[:, cc * CH:(cc + 1) * CH], pacc[:, :])
        if cc == C // CH - 1:
            nc.sync.dma_start(ap["out"][pb, row0:row0 + 128, :], st[:, :])

    def proj_tasks_of(pb, pich, yTs_p):
        tasks = []
        for pit in range(CH // 128):
            st = spool.tile([128, C], BF16, tag="ostage",
                            name=f"st_b{pb}i{pich}t{pit}")
            for cc in range(C // CH):
                tasks.append((pb, pich, pit, cc, yTs_p, st))
        return tasks

    def run_norm_a(entry):
        """Stage a: reciprocal + scale on DVE — this is what releases the
        A@V PSUM banks for the next head."""
        pb, tgt, pich, h, it, yus = entry
        rec = rpool.tile([128, 1], F32, tag="rec", name="rec")
        nc.vector.reciprocal(rec[:, :], yus[:, D:D + 1])
        yn = tpool.tile([128, 128], BF16, tag="yn", bufs=8,
                        name=f"yn_b{pb}h{h}i{pich}t{it}")
        nc.vector.tensor_scalar_mul(yn[:, :], yus[:, 0:D], rec[:, 0:1])
        normb_q.append((pb, tgt, pich, h, it, yn))

    def run_norm_b(entry):
        """Stage b: PE transpose + staging copy, two j-pairs later so the
        DVE chain from stage a never stalls the PE."""
        pb, tgt, pich, h, it, yn = entry
        tp = pspool.tile([128, 128], BF16, tag="sm", bufs=2,
                         name=f"tp_b{pb}h{h}i{pich}t{it}")
        nc.tensor.transpose(tp[:, :], yn[:, :], ident_sb[:, :])
        yt = ypool.tile([128, 128], BF16, tag="yT", name=f"yt_b{pb}h{h}i{pich}t{it}")
        nc.vector.tensor_copy(yt[:, :], tp[:, :])
        tgt[(h, it)] = yt

    for b in range(B):
        # ================= QKV projection + RoPE =================
        qT = {}
        kT = {}
        for h in range(HLOC):
            qT[h] = qkpool.tile([128, T], F32R, tag="qkT", name=f"qT_b{b}h{h}")
            kT[h] = qkpool.tile([128, T], F32R, tag="qkT", name=f"kT_b{b}h{h}")
        vaug = {}

        pre_exp = {}
        for ch in range(NCH):
            tsl = slice(ch * CH, (ch + 1) * CH)
            xt = fetch_x(b, ch)

            # --- q,k accumulation: both heads packed per [128,1024] psum pair ---
            qaccp = pspool.tile([128, 2 * CH], F32, tag="qs", bufs=2,
                                name=f"qacc_b{b}c{ch}")
            kaccp = pspool.tile([128, 2 * CH], F32, tag="qs", bufs=2,
                                name=f"kacc_b{b}c{ch}")
            for ct in range(NCT):
                xsl = xt[:, ct * CH:(ct + 1) * CH]
                for h in range(HLOC):
                    nc.tensor.matmul(
                        qaccp[:, h * CH:(h + 1) * CH],
                        wq_sb[:, ct * HD2 + h * D: ct * HD2 + (h + 1) * D],
                        xsl, start=(ct == 0), stop=(ct == NCT - 1))
                    nc.tensor.matmul(
                        kaccp[:, h * CH:(h + 1) * CH],
                        wk_sb[:, ct * HD2 + h * D: ct * HD2 + (h + 1) * D],
                        xsl, start=(ct == 0), stop=(ct == NCT - 1))

            # copy q/k out of PSUM. k first (scores need every kT tile before
            # the attention phase), except on the last chunk where the q PSUM
            # pair is what the first score tile's ring slot waits on. On chunk
            # 0 the copies go AFTER the v-loop emission so DVE prioritizes the
            # va copies the v accumulator ring is waiting on.
            order = ((kaccp, kT), (qaccp, qT))
            if ch == NCH - 1:
                order = ((qaccp, qT), (kaccp, kT))

            def emit_qk_copies():
                for (acc, tgts) in order:
                    for h in range(HLOC):
                        nc.vector.tensor_copy(tgts[h][:, tsl], acc[:, h * CH:(h + 1) * CH])

            if ch != 0:
                emit_qk_copies()

            # --- v accumulation (overlaps the rope copies on DVE); on the last
            # chunk, the first attention chunk's first score pairs + exps
            # (head 0, j-tiles 0..7: their kT/qT chunks are already roped) are
            # interleaved between v iterations so ScalarE gets a ~4-pair head
            # start on the attention phase instead of trailing it ---
            for tt in range(CH // 128):
                vacc = pspool.tile([128, HD2], F32, tag="sm", bufs=2,
                                   name=f"vacc_b{b}c{ch}t{tt}")
                for ct in range(NCT):
                    nc.tensor.matmul(
                        vacc[:, :],
                        xt[:, ct * CH + tt * 128: ct * CH + (tt + 1) * 128],
                        wv_sb[:, ct * HD2:(ct + 1) * HD2],
                        start=(ct == 0), stop=(ct == NCT - 1))
                jt = ch * (CH // 128) + tt
                for h in range(HLOC):
                    va = vpool.tile([128, 130], BF16, tag="vaug", name=f"va_b{b}h{h}j{jt}")
                    nc.vector.tensor_copy(va[:, 0:D], vacc[:, h * D:(h + 1) * D])
                    nc.gpsimd.memset(va[:, D:D + 1], 1.0)
                    vaug[(h, jt)] = va
                if ch == NCH - 1:
                    jp = tt
                    scp = pspool.tile([128, 2 * CH], F32, tag="qs", bufs=2,
                                      name=f"scp_b{b}i0h0p{jp}")
                    for k in range(2):
                        jjt = 2 * jp + k
                        nc.tensor.matmul(scp[:, k * CH:(k + 1) * CH],
                                         kT[0][:, jjt * 128:(jjt + 1) * 128],
                                         qT[0][:, 0:CH],
                                         start=True, stop=True)
                    ep = epool.tile([128, 2 * CH], BF16, tag="expT",
                                    name=f"e_b{b}i0h0p{jp}")
                    nc.scalar.activation(ep[:, :], scp[:, :], AF.Exp, bias=zbias[:, 0:1])
                    pre_exp[jp] = ep

            if ch == 0:
                emit_qk_copies()

            # --- RoPE rotate (k first) ---
            for tgts in (kT, qT):
                for h in range(HLOC):
                    tgt = tgts[h]
                    shuf = pspool.tile([ROT, CH], F32, tag="sm", bufs=2,
                                       name=f"shuf_b{b}c{ch}h{h}")
                    nc.tensor.matmul(shuf[:, :], perm_sb[:, :], tgt[0:ROT, tsl],
                                     start=True, stop=True)
                    tmp1 = tpool.tile([ROT, CH], F32, tag="tmp", name="rtmp1")
                    tmp2 = tpool.tile([ROT, CH], F32, tag="tmp", name="rtmp2")
                    nc.vector.tensor_mul(tmp1[:, :], shuf[:, :], sin_sb[:, tsl])
                    nc.vector.tensor_mul(tmp2[:, :], tgt[0:ROT, tsl], cos_sb[:, tsl])
                    nc.vector.tensor_add(tgt[0:ROT, tsl], tmp1[:, :], tmp2[:, :])

            if b == 0 and ch == min(1, NCH - 1):
                nc.sync.dma_start(wp_sb[:, :], ap["wp"][:, :])

        # ================= attention + projection =================
        for ich in range(NCH):
            isl = slice(ich * CH, (ich + 1) * CH)
            # prefetch next batch's first x chunks during this attention phase
            if b == 0 and ich in (0, 1) and B > 1:
                xt = xpool.tile([128, XW], BF16, tag="x", name=f"x_b1c{ich}")
                nc.sync.dma_start(xt[:, :], ap["xT"][1, ich, :, :])
                x_pre[(1, ich)] = xt

            pending = proj_tasks_of(*proj_prev) if proj_prev is not None else []
            pending.reverse()  # pop() from the front in order
            yTs = {}
            NIT = CH // 128
            yu_of = {}

            NPJ = NJT // 2  # j-tile pairs
            PLAG = 4        # pairs between a score pair and its A@V consumers
            for h in range(HLOC):
                # scores and A@V interleaved per j-tile pair: two score matmuls
                # fill halves of one [128,1024] PSUM tile, ONE exp covers both
                # (the ScalarE per-instruction init is the attention phase's
                # co-critical cost). PSUM zero regions are bank-granular, so
                # only 2 A@V accumulation groups can be live (one per "yu"
                # bank): pass 1 accumulates i-tiles 0,1 alongside the score
                # stream, pass 2 accumulates i-tiles 2,3 from the SBUF exp
                # tiles afterwards.
                expp = {}
                for it in (0, 1):
                    yu = pspool.tile([128, 512], F32, tag="yu", bufs=2,
                                     name=f"yu_b{b}h{h}i{ich}t{it}")
                    yu_of[(h, it)] = yu

                for jp in range(NPJ + PLAG):
                    if jp < NPJ:
                        if ich == 0 and h == 0 and jp in pre_exp:
                            expp[jp] = pre_exp.pop(jp)
                        else:
                            scp = pspool.tile([128, 2 * CH], F32, tag="qs", bufs=2,
                                              name=f"scp_b{b}i{ich}h{h}p{jp}")
                            for k in range(2):
                                jt = 2 * jp + k
                                nc.tensor.matmul(scp[:, k * CH:(k + 1) * CH],
                                                 kT[h][:, jt * 128:(jt + 1) * 128],
                                                 qT[h][:, isl],
                                                 start=True, stop=True)
                            ep = epool.tile([128, 2 * CH], BF16, tag="expT",
                                            name=f"e_b{b}i{ich}h{h}p{jp}")
                            nc.scalar.activation(ep[:, :], scp[:, :], AF.Exp, bias=zbias[:, 0:1])
                            expp[jp] = ep
                    # drain deferred normalizes (the previous head's pass-2
                    # accumulators): stage a at jp=0,1 frees those banks for
                    # this head's pass-1 writes at jp=PLAG; stage b follows
                    # when the DVE chain has certainly finished
                    if jp in (0, 1):
                        if norm_q:
                            run_norm_a(norm_q.pop(0))
                    if jp in (1, 2):
                        if normb_q:
                            run_norm_b(normb_q.pop(0))
                    # previous i-chunk's projection as PE gap-filler: keeps the
                    # PE ahead of ScalarE's exp stream instead of stalling on it
                    if pending and 2 <= jp:
                        emit_proj_task(pending.pop())
                    if jp >= PLAG:
                        p = jp - PLAG
                        for k in range(2):
                            j = 2 * p + k
                            for it in (0, 1):
                                nc.tensor.matmul(yu_of[(h, it)][:, 0:D + 1],
                                                 expp[p][:, k * CH + it * 128:k * CH + (it + 1) * 128],
                                                 vaug[(h, j)][:, 0:D + 1],
                                                 start=(j == 0), stop=(j == NJT - 1))
                # pass 2: i-tiles 2,3 accumulate from the SBUF exp tiles while
                # the normalize/projection fillers keep the other engines fed
                for it in (2, 3):
                    yu = pspool.tile([128, 512], F32, tag="yu", bufs=2,
                                     name=f"yu_b{b}h{h}i{ich}t{it}")
                    yu_of[(h, it)] = yu
                for q in range(NPJ):
                    if q == 0:
                        run_norm_a((b, yTs, ich, h, 0, yu_of[(h, 0)]))
                        run_norm_a((b, yTs, ich, h, 1, yu_of[(h, 1)]))
                    if q in (2, 3):
                        run_norm_b(normb_q.pop(0))
                    for k in range(2):
                        j = 2 * q + k
                        for it in (2, 3):
                            nc.tensor.matmul(yu_of[(h, it)][:, 0:D + 1],
                                             expp[q][:, k * CH + it * 128:k * CH + (it + 1) * 128],
                                             vaug[(h, j)][:, 0:D + 1],
                                             start=(j == 0), stop=(j == NJT - 1))
                # pass-2 normalizes run inside the next head's (or chunk's) bursts
                for it in (2, 3):
                    norm_q.append((b, yTs, ich, h, it, yu_of[(h, it)]))
            while pending:
                emit_proj_task(pending.pop())
            proj_prev = (b, ich, yTs)

        # trailing work for this batch's last i-chunk: pit0/pit1 tasks have
        # ready yT tiles, so they lead off the PE stream while the norm
        # chain for (h1,it2),(h1,it3) drains on DVE in parallel; on the
        # final batch, the staging copies alternate DVE/Act and the last
        # row block DMAs per column block to shorten the end drain
        tail_tasks = proj_tasks_of(*proj_prev)
        for it in range(CH // 128):
            for i, task in enumerate(tail_tasks[4 * it:4 * it + 4]):
                if norm_q and i in (0, 1):
                    run_norm_a(norm_q.pop(0))
                if normb_q and i in (2, 3):
                    run_norm_b(normb_q.pop(0))
                emit_proj_task(task, tail=(b == B - 1),
                               final=(b == B - 1 and it == CH // 128 - 1))
        proj_prev = None

    ctx.close()


def make_nc(B=2, T=2048, C=2048, reps=1, loop=1):
    nc = bacc.Bacc("TRN2", target_bir_lowering=False, debug=False)
    nc.tensor_map = {}

    def dram(name, shape, kind, dt=F32):
        t = nc.dram_tensor(name, shape, dt, kind=kind)
        nc.tensor_map[name] = t
        return t

    CH = 512
    NCH = T // CH
    NCT = C // 128
    dram("xT", [B, NCH, 128, NCT * CH], "ExternalInput", BF16)
    dram("wq", [128, NCT * HLOC * D], "ExternalInput", BF16)
    dram("wk", [128, NCT * HLOC * D], "ExternalInput", BF16)
    dram("wv", [128, NCT * HLOC * D], "ExternalInput", BF16)
    dram("wp", [128, HLOC * T], "ExternalInput", BF16)
    dram("cos_e", [ROT, T], "ExternalInput")
    dram("sin_e", [ROT, T], "ExternalInput")
    dram("perm", [ROT, ROT], "ExternalInput", F32R)
    dram("ident", [128, 128], "ExternalInput", BF16)
    dram("out", [B, T, C], "ExternalOutput", BF16)

    with tile.TileContext(nc) as tc:
        if loop > 1:
            with tc.For_i(0, loop, 1):
                build_core_kernel(nc, tc, B, T, C)
        else:
            for _ in range(reps):
                build_core_kernel(nc, tc, B, T, C)
    nc.compile()
    return nc


def host_inputs(x, W_attn, W_proj, cos, sin):
    """Build the 8 per-core input maps from the full-problem inputs."""
    import ml_dtypes
    BF = ml_dtypes.bfloat16
    B, T, C = x.shape
    CH = 512
    NCH = T // CH
    NCT = C // 128
    # xr[b, ch, p, ct*CH + c] = x[b, ch*CH + c, ct*128 + p]
    xr = np.ascontiguousarray(
        x.reshape(B, NCH, CH, NCT, 128).transpose(0, 1, 4, 3, 2)
        .reshape(B, NCH, 128, NCT * CH)).astype(BF)
    cos_e = np.repeat(np.ascontiguousarray(cos.T), 2, axis=0).astype(np.float32)
    sin_e = np.repeat(np.ascontiguousarray(sin.T), 2, axis=0).astype(np.float32)
    sin_e[0::2, :] *= -1.0
    perm = np.zeros((ROT, ROT), np.float32)
    for i in range(ROT):
        perm[i, i ^ 1] = 1.0
    ident = np.eye(128, dtype=np.float32).astype(BF)
    scale = 1.0 / math.sqrt(D)

    def wlayout(w):
        # [C, HLOC*D] -> [128, NCT * HLOC*D] with ct-major column blocks
        return np.ascontiguousarray(
            w.reshape(NCT, 128, HLOC * D).transpose(1, 0, 2).reshape(128, -1)).astype(BF)

    in_maps = []
    for core in range(N_CORES):
        h0 = core * HLOC
        sl = slice(h0 * D, (h0 + HLOC) * D)
        wp = np.ascontiguousarray(
            W_proj[sl, :].reshape(HLOC, 128, C).transpose(1, 0, 2).reshape(128, -1)).astype(BF)
        in_maps.append({
            "xT": xr,
            "wq": wlayout(W_attn[:, sl] * scale),
            "wk": wlayout(W_attn[:, C + h0 * D: C + (h0 + HLOC) * D]),
            "wv": wlayout(W_attn[:, 2 * C + h0 * D: 2 * C + (h0 + HLOC) * D]),
            "wp": wp,
            "cos_e": cos_e,
            "sin_e": sin_e,
            "perm": perm,
            "ident": ident,
        })
    return in_maps


_NC_CACHE = {}


def build_runner(nc):
    """Build a sharded jitted runner over 8 cores for an arbitrary nc, modeled
    on concourse.bass2jax.run_bass_via_pjrt but with a cached executable."""
    import jax
    from jax.sharding import Mesh, PartitionSpec
    from jax.experimental.shard_map import shard_map
    from concourse import bass2jax

    bass2jax.install_neuronx_cc_hook()

    partition_name = nc.partition_id_tensor.name if nc.partition_id_tensor else None
    in_names, out_names, out_avals, zero_shapes = [], [], [], []
    for alloc in nc.m.functions[0].allocations:
        if not isinstance(alloc, mybir.MemoryLocationSet):
            continue
        name = alloc.memorylocations[0].name
        if alloc.kind == "ExternalInput":
            if name != partition_name:
                in_names.append(name)
        elif alloc.kind == "ExternalOutput":
            out_names.append(name)
            shape = tuple(alloc.tensor_shape)
            dtype = mybir.dt.np(alloc.dtype)
            out_avals.append(jax.core.ShapedArray(shape, dtype))
            zero_shapes.append((shape, dtype))
    n_params = len(in_names)
    n_outs = len(out_names)
    all_names = in_names + out_names
    if partition_name is not None:
        all_names = all_names + [partition_name]

    def _body(*args):
        operands = list(args)
        if partition_name is not None:
            operands.append(bass2jax.partition_id_tensor())
        outs = bass2jax._bass_exec_p.bind(
            *operands,
            out_avals=tuple(out_avals),
            in_names=tuple(all_names),
            out_names=tuple(out_names),
            lowering_input_output_aliases=(),
            sim_require_finite=True,
            sim_require_nnan=True,
            nc=nc,
        )
        return tuple(outs)

    devices = jax.devices()[:N_CORES]
    mesh = Mesh(np.asarray(devices), ("core",))
    in_specs = (PartitionSpec("core"),) * (n_params + n_outs)
    out_specs = (PartitionSpec("core"),) * n_outs
    donate = tuple(range(n_params, n_params + n_outs))
    sharded = jax.jit(
        shard_map(_body, mesh=mesh, in_specs=in_specs, out_specs=out_specs,
                  check_rep=False),
        donate_argnums=donate, keep_unused=True)

    runner = {
        "fn": sharded, "in_names": in_names, "out_names": out_names,
        "zero_shapes": zero_shapes, "n_params": n_params, "mesh": mesh,
    }
    return runner


def _get_runner(reps=1, loop=1):
    key = ("runner", reps, loop)
    if key not in _NC_CACHE:
        _NC_CACHE[key] = build_runner(make_nc(reps=reps, loop=loop))
    return _NC_CACHE[key]


def _concat_inputs(runner, in_maps):
    return [np.concatenate([in_maps[c][name] for c in range(N_CORES)], axis=0)
            for name in runner["in_names"]]


def _make_zeros(runner):
    return [np.zeros((N_CORES * s[0], *s[1:]), dt)
            for (s, dt) in runner["zero_shapes"]]


def kernel(x, W_attn, W_proj, cos, sin):
    x = np.asarray(x, np.float32)
    W_attn = np.asarray(W_attn, np.float32)
    W_proj = np.asarray(W_proj, np.float32)
    cos = np.asarray(cos, np.float32)
    sin = np.asarray(sin, np.float32)

    runner = _get_runner()
    in_maps = host_inputs(x, W_attn, W_proj, cos, sin)
    out_arrs = runner["fn"](*_concat_inputs(runner, in_maps), *_make_zeros(runner))
    B, T, C = x.shape
    parts = np.asarray(out_arrs[0]).astype(np.float32).reshape(N_CORES, B, T, C)
    return parts.sum(axis=0, dtype=np.float32)


def bench(x, W_attn, W_proj, cos, sin, iters=10, reps=1, loop=1):
    """Time device-resident executions; returns (output, per-iter seconds list)."""
    import time
    import jax
    from jax.sharding import NamedSharding, PartitionSpec

    runner = _get_runner(reps=reps, loop=loop)
    in_maps = host_inputs(np.asarray(x, np.float32), np.asarray(W_attn, np.float32),
                          np.asarray(W_proj, np.float32), np.asarray(cos, np.float32),
                          np.asarray(sin, np.float32))
    sharding = NamedSharding(runner["mesh"], PartitionSpec("core"))
    dev_in = [jax.device_put(a, sharding) for a in _concat_inputs(runner, in_maps)]
    zero_sets = [[jax.device_put(z, sharding) for z in _make_zeros(runner)]
                 for _ in range(iters + 1)]
    for z in zero_sets:
        jax.block_until_ready(z)
    jax.block_until_ready(dev_in)

    # warmup (also compiles)
    out = runner["fn"](*dev_in, *zero_sets[0])
    jax.block_until_ready(out)
    times = []
    for i in range(iters):
        t0 = time.perf_counter()
        out = runner["fn"](*dev_in, *zero_sets[i + 1])
        jax.block_until_ready(out)
        times.append(time.perf_counter() - t0)
    B, T, C = np.asarray(x).shape
    parts = np.asarray(out[0]).astype(np.float32).reshape(N_CORES, B, T, C)
    return parts.sum(axis=0, dtype=np.float32), times


if __name__ == "__main__":
    np.random.seed(0)
    B, T, C = 2, 2048, 2048
    x = np.random.randn(B, T, C).astype(np.float32)
    W_attn = (np.random.randn(C, 3 * C) / math.sqrt(C)).astype(np.float32)
    W_proj = (np.random.randn(C, C) / math.sqrt(C)).astype(np.float32)
    half = ROT // 2
    inv = 1.0 / 10000.0 ** (np.arange(half, dtype=np.float32) / half)
    ang = np.outer(np.arange(T, dtype=np.float32), inv)
    out = kernel(x, W_attn, W_proj, np.cos(ang), np.sin(ang))
    print(out.shape, out.dtype)



# revision 22
# speedup vs baseline: 2.2785x; 1.0720x over previous
"""Bidirectional multi-head self-attention (B=2, T=2048, C=2048, H=16, D=128,
partial RoPE over first 64 dims) on 8 TRN2 NeuronCores.

Sharding: tensor-parallel over heads. Core c computes heads (2c, 2c+1) for both
batches: qkv projection with the corresponding W_attn column slices, attention,
and the partial output  y_heads @ W_proj[head_rows, :].  The 8 partial [B,T,C]
outputs (bf16) are summed on the host (W_proj mixes heads into every output
column).

Per-core kernel layout choices:
  - x / W_attn / W_proj are fed in bf16 (PE rate is identical to fp32r, halves
    DMA bytes); intermediates (qT/kT, scores) stay fp32r.
  - x is fed transposed and chunk-blocked on the host so ONE DMA delivers a
    whole [128, 16*512] chunk (HWDGE serializes ~625ns per DMA descriptor
    chain, so DMA COUNT -- not bytes -- is the startup bottleneck).
  - RoPE: pair-swap via a 64x64 permutation matmul on TensorE + cos/sin
    elementwise combines on VectorE (cos/sin tables pre-expanded on host,
    sign folded into the sin table); 1/sqrt(D) folded into W_q on host.
  - scores are computed transposed (scT[j,i] = lhsT=kT tile, rhs=qT chunk)
    in fp32r, one [128,512] PSUM bank per (j-tile, head).
  - softmax: no max subtraction needed (scores ~ N(0,1)); exp on ScalarE
    PSUM->SBUF (bf16); denominator fused into the A@V matmul via an extra
    all-ones column appended to V.
  - A@V in bf16, interleaved with the score/exp stream per j-tile pair (two
    passes of 2 i-tiles each: PSUM zero regions are bank-granular, so only two
    accumulation groups can be live in the 2 "yu" banks); normalize on
    VectorE; transpose y via TensorE (bf16); project with W_proj slices as
    deferred PE gap-filler inside the next i-chunk's stream; outputs staged to
    [128, 2048] bf16 tiles, one DMA per 128-row block.

PSUM (8 banks = 16KB/partition): tag "qs" 2x[128,1024] (q/k accumulator pairs
in the qkv phase, score j-tile pairs in attention), tag "sm" 2x[128,512]
(v accumulators / rope shuffle / y transpose / projection accumulators), tag
"yu" 2x[128,512] A@V accumulator banks.
"""

import math
import numpy as np

from concourse import bass, bacc, mybir, tile
from concourse.bass_utils import run_bass_kernel_spmd

F32 = mybir.dt.float32
F32R = mybir.dt.float32r
BF16 = mybir.dt.bfloat16
AF = mybir.ActivationFunctionType
AO = mybir.AluOpType

N_CORES = 8
N_HEAD = 16
ROT = 64  # rotary dims per head
D = 128   # head dim
HLOC = N_HEAD // N_CORES  # heads per core = 2


def build_core_kernel(nc, tc, B, T, C):
    """Emit the per-core program. All DRAM tensors are declared on `nc` before
    the TileContext is entered; this emits into `tc`."""
    CH = 512            # t-chunk size (qkv chunks and attention i-chunks)
    NCH = T // CH
    NCT = C // 128      # contraction tiles over C
    NJT = T // 128      # j (key) tiles
    HD2 = HLOC * D      # 256
    XW = NCT * CH       # x chunk width in sbuf cols

    ap = {name: nc.tensor_map[name].ap() for name in
          ("xT", "wq", "wk", "wv", "wp", "cos_e", "sin_e", "perm", "ident", "out")}

    from contextlib import ExitStack
    ctx = ExitStack()

    wpool = ctx.enter_context(tc.tile_pool(name="wpool", bufs=1))
    xpool = ctx.enter_context(tc.tile_pool(name="xpool", bufs=3))
    qkpool = ctx.enter_context(tc.tile_pool(name="qkpool", bufs=4))
    vpool = ctx.enter_context(tc.tile_pool(name="vpool", bufs=32))
    epool = ctx.enter_context(tc.tile_pool(name="epool", bufs=12))
    ypool = ctx.enter_context(tc.tile_pool(name="ypool", bufs=9))
    spool = ctx.enter_context(tc.tile_pool(name="spool", bufs=3))
    tpool = ctx.enter_context(tc.tile_pool(name="tpool", bufs=2))
    rpool = ctx.enter_context(tc.tile_pool(name="rpool", bufs=4))
    pspool = ctx.enter_context(tc.tile_pool(name="pspool", bufs=2, space="PSUM"))

    # ---- static weights/tables + first-chunk x, interleaved for fast start ----
    wq_sb = wpool.tile([128, NCT * HD2], BF16, tag="wq")
    wk_sb = wpool.tile([128, NCT * HD2], BF16, tag="wk")
    wv_sb = wpool.tile([128, NCT * HD2], BF16, tag="wv")
    wp_sb = wpool.tile([128, HLOC * T], BF16, tag="wp")

    x_first = xpool.tile([128, XW], BF16, tag="x", name="x_b0c0")
    QW = XW // 4
    HW = NCT * HD2 // 2
    # interleave: x quarter, then a weight half, so the first q/k matmuls can
    # start after ~3 transfers instead of after the full weight set
    cos_sb = wpool.tile([ROT, T], F32, tag="cos")
    sin_sb = wpool.tile([ROT, T], F32, tag="sin")
    perm_sb = wpool.tile([ROT, ROT], BF16, tag="perm")
    ident_sb = wpool.tile([128, 128], BF16, tag="ident")

    def _pieces(total, sizes):
        o, out = 0, []
        for s in sizes:
            out.append((o, o + s))
            o += s
        assert o == total
        return out

    # startup order tuned for the serial ~625ns/DMA HWDGE setup chain: the
    # first q/k matmuls need only (x 1-ct piece, wq 1-ct piece, wk 1-ct
    # piece); x(0,1) is prefetched at the end of the chain (needed ~20us in),
    # ident (first used in the attention phase ~90us in) goes last
    xp = _pieces(XW, [2 * CH, 2 * CH, 4 * CH, 4 * CH, 4 * CH])
    wqp = _pieces(NCT * HD2, [4 * HD2, 12 * HD2])
    wvp = _pieces(NCT * HD2, [8 * HD2, 8 * HD2])
    x01p = _pieces(XW, [8 * CH, 8 * CH])
    x_next = xpool.tile([128, XW], BF16, tag="x", name="x_b0c1")
    seqs = [
        ("x", xp[0]), ("wq", wqp[0]), ("wk", wqp[0]), ("x", xp[1]),
        ("perm", None), ("wq", wqp[1]), ("wk", wqp[1]), ("x", xp[2]),
        ("cos", None), ("wv", wvp[0]), ("x", xp[3]), ("sin", None),
        ("wv", wvp[1]), ("x", xp[4]),
        ("x01", x01p[0]), ("x01", x01p[1]), ("ident", None),
    ]
    for kind, pc in seqs:
        if kind == "x":
            nc.sync.dma_start(x_first[:, pc[0]:pc[1]], ap["xT"][0, 0, :, pc[0]:pc[1]])
        elif kind == "x01":
            nc.sync.dma_start(x_next[:, pc[0]:pc[1]], ap["xT"][0, 1, :, pc[0]:pc[1]])
        elif kind in ("wq", "wk", "wv"):
            sb = {"wq": wq_sb, "wk": wk_sb, "wv": wv_sb}[kind]
            nc.sync.dma_start(sb[:, pc[0]:pc[1]], ap[kind][:, pc[0]:pc[1]])
        elif kind == "perm":
            nc.sync.dma_start(perm_sb[:, :], ap["perm"][:, :])
        elif kind == "cos":
            nc.sync.dma_start(cos_sb[:, :], ap["cos_e"][:, :])
        elif kind == "sin":
            nc.sync.dma_start(sin_sb[:, :], ap["sin_e"][:, :])
        elif kind == "ident":
            nc.sync.dma_start(ident_sb[:, :], ap["ident"][:, :])
    zbias = wpool.tile([128, 1], F32, tag="zbias")
    nc.gpsimd.memset(zbias[:, :], 0.0)

    x_pre = {(0, 0): x_first, (0, 1): x_next}

    def fetch_x(b, ch):
        if (b, ch) in x_pre:
            return x_pre.pop((b, ch))
        xt = xpool.tile([128, XW], BF16, tag="x", name=f"x_b{b}c{ch}")
        nc.sync.dma_start(xt[:, :], ap["xT"][b, ch, :, :])
        return xt

    # Projection/normalize pipeline state carried ACROSS i-chunks and across
    # the batch boundary: batch b's first i-chunk projects batch b-1's last
    # i-chunk as PE gap-filler (the baseline drained it in a serial tail).
    proj_prev = None  # (pb, pich, yTs) of the previous i-chunk
    norm_q = []   # A@V accumulators awaiting normalize (rec+yn on DVE)
    normb_q = []  # normalized yn tiles awaiting transpose (PE) + copy

    def emit_proj_task(task, tail=False, final=False):
        """One projection column block: 2 accumulating matmuls + staging
        copy; fires the row-block DMA when its last column lands.  In the
        end-of-kernel tail the staging copy alternates DVE/Act (the tail is
        DVE-queue-bound) and the last row block DMAs per column block to
        shorten the drain."""
        pb, pich, pit, cc, yTs_p, st = task
        row0 = pich * CH + pit * 128
        ptag = "yu" if (tail and pit >= 2 and cc % 2 == 1) else "sm"
        pacc = pspool.tile([128, CH], F32, tag=ptag, bufs=2,
                           name=f"pacc_b{pb}i{pich}t{pit}c{cc}")
        for hh in range(HLOC):
            nc.tensor.matmul(pacc[:, :],
                             yTs_p[(hh, pit)][:, :],
                             wp_sb[:, hh * T + cc * CH:hh * T + (cc + 1) * CH],
                             start=(hh == 0), stop=(hh == HLOC - 1))
        if tail and cc % 2 == 1:
            nc.scalar.copy(st[:, cc * CH:(cc + 1) * CH], pacc[:, :])
        else:
            nc.vector.tensor_copy(st[:, cc * CH:(cc + 1) * CH], pacc[:, :])
        if cc == C // CH - 1:
            nc.sync.dma_start(ap["out"][pb, row0:row0 + 128, :], st[:, :])

    def proj_tasks_of(pb, pich, yTs_p):
        tasks = []
        for pit in range(CH // 128):
            st = spool.tile([128, C], BF16, tag="ostage",
                            name=f"st_b{pb}i{pich}t{pit}")
            for cc in range(C // CH):
                tasks.append((pb, pich, pit, cc, yTs_p, st))
        return tasks

    def run_norm_a(entry):
        """Stage a: reciprocal + scale on DVE — this is what releases the
        A@V PSUM banks for the next head."""
        pb, tgt, pich, h, it, yus = entry
        rec = rpool.tile([128, 1], F32, tag="rec", name="rec")
        nc.vector.reciprocal(rec[:, :], yus[:, D:D + 1])
        yn = tpool.tile([128, 128], BF16, tag="yn", bufs=8,
                        name=f"yn_b{pb}h{h}i{pich}t{it}")
        nc.vector.tensor_scalar_mul(yn[:, :], yus[:, 0:D], rec[:, 0:1])
        normb_q.append((pb, tgt, pich, h, it, yn))

    def run_norm_b(entry):
        """Stage b: PE transpose + staging copy, two j-pairs later so the
        DVE chain from stage a never stalls the PE."""
        pb, tgt, pich, h, it, yn = entry
        tp = pspool.tile([128, 128], BF16, tag="sm", bufs=2,
                         name=f"tp_b{pb}h{h}i{pich}t{it}")
        nc.tensor.transpose(tp[:, :], yn[:, :], ident_sb[:, :])
        yt = ypool.tile([128, 128], BF16, tag="yT", name=f"yt_b{pb}h{h}i{pich}t{it}")
        nc.vector.tensor_copy(yt[:, :], tp[:, :])
        tgt[(h, it)] = yt

    for b in range(B):
        # ================= QKV projection + RoPE =================
        qT = {}
        kT = {}
        for h in range(HLOC):
            qT[h] = qkpool.tile([128, T], BF16, tag="qkT", name=f"qT_b{b}h{h}")
            kT[h] = qkpool.tile([128, T], BF16, tag="qkT", name=f"kT_b{b}h{h}")
        vaug = {}

        pre_exp = {}
        for ch in range(NCH):
            tsl = slice(ch * CH, (ch + 1) * CH)
            xt = fetch_x(b, ch)

            # --- q,k accumulation: both heads packed per [128,1024] psum pair ---
            qaccp = pspool.tile([128, 2 * CH], F32, tag="qs", bufs=2,
                                name=f"qacc_b{b}c{ch}")
            kaccp = pspool.tile([128, 2 * CH], F32, tag="qs", bufs=2,
                                name=f"kacc_b{b}c{ch}")
            for ct in range(NCT):
                xsl = xt[:, ct * CH:(ct + 1) * CH]
                for h in range(HLOC):
                    nc.tensor.matmul(
                        qaccp[:, h * CH:(h + 1) * CH],
                        wq_sb[:, ct * HD2 + h * D: ct * HD2 + (h + 1) * D],
                        xsl, start=(ct == 0), stop=(ct == NCT - 1))
                    nc.tensor.matmul(
                        kaccp[:, h * CH:(h + 1) * CH],
                        wk_sb[:, ct * HD2 + h * D: ct * HD2 + (h + 1) * D],
                        xsl, start=(ct == 0), stop=(ct == NCT - 1))

            # copy q/k out of PSUM. k first (scores need every kT tile before
            # the attention phase), except on the last chunk where the q PSUM
            # pair is what the first score tile's ring slot waits on. On chunk
            # 0 the copies go AFTER the v-loop emission so DVE prioritizes the
            # va copies the v accumulator ring is waiting on.
            order = ((kaccp, kT), (qaccp, qT))
            if ch == NCH - 1:
                order = ((qaccp, qT), (kaccp, kT))

            def emit_qk_copies():
                for (acc, tgts) in order:
                    for h in range(HLOC):
                        nc.vector.tensor_copy(tgts[h][:, tsl], acc[:, h * CH:(h + 1) * CH])

            if ch != 0:
                emit_qk_copies()

            # --- v accumulation (overlaps the rope copies on DVE); on the last
            # chunk, the first attention chunk's first score pairs + exps
            # (head 0, j-tiles 0..7: their kT/qT chunks are already roped) are
            # interleaved between v iterations so ScalarE gets a ~4-pair head
            # start on the attention phase instead of trailing it ---
            for tt in range(CH // 128):
                vacc = pspool.tile([128, HD2], F32, tag="sm", bufs=2,
                                   name=f"vacc_b{b}c{ch}t{tt}")
                for ct in range(NCT):
                    nc.tensor.matmul(
                        vacc[:, :],
                        xt[:, ct * CH + tt * 128: ct * CH + (tt + 1) * 128],
                        wv_sb[:, ct * HD2:(ct + 1) * HD2],
                        start=(ct == 0), stop=(ct == NCT - 1))
                jt = ch * (CH // 128) + tt
                for h in range(HLOC):
                    va = vpool.tile([128, 130], BF16, tag="vaug", name=f"va_b{b}h{h}j{jt}")
                    nc.vector.tensor_copy(va[:, 0:D], vacc[:, h * D:(h + 1) * D])
                    nc.gpsimd.memset(va[:, D:D + 1], 1.0)
                    vaug[(h, jt)] = va
                if ch == NCH - 1:
                    jp = tt
                    scp = pspool.tile([128, 2 * CH], F32, tag="qs", bufs=2,
                                      name=f"scp_b{b}i0h0p{jp}")
                    for k in range(2):
                        jjt = 2 * jp + k
                        nc.tensor.matmul(scp[:, k * CH:(k + 1) * CH],
                                         kT[0][:, jjt * 128:(jjt + 1) * 128],
                                         qT[0][:, 0:CH],
                                         start=True, stop=True)
                    ep = epool.tile([128, 2 * CH], BF16, tag="expT",
                                    name=f"e_b{b}i0h0p{jp}")
                    nc.scalar.activation(ep[:, :], scp[:, :], AF.Exp, bias=zbias[:, 0:1])
                    pre_exp[jp] = ep

            if ch == 0:
                emit_qk_copies()

            # --- RoPE rotate (k first) ---
            for tgts in (kT, qT):
                for h in range(HLOC):
                    tgt = tgts[h]
                    shuf = pspool.tile([ROT, CH], F32, tag="sm", bufs=2,
                                       name=f"shuf_b{b}c{ch}h{h}")
                    nc.tensor.matmul(shuf[:, :], perm_sb[:, :], tgt[0:ROT, tsl],
                                     start=True, stop=True)
                    tmp1 = tpool.tile([ROT, CH], F32, tag="tmp", name="rtmp1")
                    tmp2 = tpool.tile([ROT, CH], F32, tag="tmp", name="rtmp2")
                    nc.vector.tensor_mul(tmp1[:, :], shuf[:, :], sin_sb[:, tsl])
                    nc.vector.tensor_mul(tmp2[:, :], tgt[0:ROT, tsl], cos_sb[:, tsl])
                    nc.vector.tensor_add(tgt[0:ROT, tsl], tmp1[:, :], tmp2[:, :])

            if b == 0 and ch == min(1, NCH - 1):
                nc.sync.dma_start(wp_sb[:, :], ap["wp"][:, :])

        # ================= attention + projection =================
        for ich in range(NCH):
            isl = slice(ich * CH, (ich + 1) * CH)
            # prefetch next batch's first x chunks during this attention phase
            if b == 0 and ich in (0, 1) and B > 1:
                xt = xpool.tile([128, XW], BF16, tag="x", name=f"x_b1c{ich}")
                nc.sync.dma_start(xt[:, :], ap["xT"][1, ich, :, :])
                x_pre[(1, ich)] = xt

            pending = proj_tasks_of(*proj_prev) if proj_prev is not None else []
            pending.reverse()  # pop() from the front in order
            yTs = {}
            NIT = CH // 128
            yu_of = {}

            NPJ = NJT // 2  # j-tile pairs
            PLAG = 4        # pairs between a score pair and its A@V consumers
            for h in range(HLOC):
                # scores and A@V interleaved per j-tile pair: two score matmuls
                # fill halves of one [128,1024] PSUM tile, ONE exp covers both
                # (the ScalarE per-instruction init is the attention phase's
                # co-critical cost). PSUM zero regions are bank-granular, so
                # only 2 A@V accumulation groups can be live (one per "yu"
                # bank): pass 1 accumulates i-tiles 0,1 alongside the score
                # stream, pass 2 accumulates i-tiles 2,3 from the SBUF exp
                # tiles afterwards.
                expp = {}
                for it in (0, 1):
                    yu = pspool.tile([128, 512], F32, tag="yu", bufs=2,
                                     name=f"yu_b{b}h{h}i{ich}t{it}")
                    yu_of[(h, it)] = yu

                for jp in range(NPJ + PLAG):
                    if jp < NPJ:
                        if ich == 0 and h == 0 and jp in pre_exp:
                            expp[jp] = pre_exp.pop(jp)
                        else:
                            scp = pspool.tile([128, 2 * CH], F32, tag="qs", bufs=2,
                                              name=f"scp_b{b}i{ich}h{h}p{jp}")
                            for k in range(2):
                                jt = 2 * jp + k
                                nc.tensor.matmul(scp[:, k * CH:(k + 1) * CH],
                                                 kT[h][:, jt * 128:(jt + 1) * 128],
                                                 qT[h][:, isl],
                                                 start=True, stop=True)
                            ep = epool.tile([128, 2 * CH], BF16, tag="expT",
                                            name=f"e_b{b}i{ich}h{h}p{jp}")
                            nc.scalar.activation(ep[:, :], scp[:, :], AF.Exp, bias=zbias[:, 0:1])
                            expp[jp] = ep
                    # drain deferred normalizes (the previous head's pass-2
                    # accumulators): stage a at jp=0,1 frees those banks for
                    # this head's pass-1 writes at jp=PLAG; stage b follows
                    # when the DVE chain has certainly finished
                    if jp in (0, 1):
                        if norm_q:
                            run_norm_a(norm_q.pop(0))
                    if jp in (1, 2):
                        if normb_q:
                            run_norm_b(normb_q.pop(0))
                    # previous i-chunk's projection as PE gap-filler: keeps the
                    # PE ahead of ScalarE's exp stream instead of stalling on it
                    if pending and 2 <= jp:
                        emit_proj_task(pending.pop())
                    if jp >= PLAG:
                        p = jp - PLAG
                        for k in range(2):
                            j = 2 * p + k
                            for it in (0, 1):
                                nc.tensor.matmul(yu_of[(h, it)][:, 0:D + 1],
                                                 expp[p][:, k * CH + it * 128:k * CH + (it + 1) * 128],
                                                 vaug[(h, j)][:, 0:D + 1],
                                                 start=(j == 0), stop=(j == NJT - 1))
                # pass 2: i-tiles 2,3 accumulate from the SBUF exp tiles while
                # the normalize/projection fillers keep the other engines fed
                for it in (2, 3):
                    yu = pspool.tile([128, 512], F32, tag="yu", bufs=2,
                                     name=f"yu_b{b}h{h}i{ich}t{it}")
                    yu_of[(h, it)] = yu
                for q in range(NPJ):
                    if q == 0:
                        run_norm_a((b, yTs, ich, h, 0, yu_of[(h, 0)]))
                        run_norm_a((b, yTs, ich, h, 1, yu_of[(h, 1)]))
                    if q in (2, 3):
                        run_norm_b(normb_q.pop(0))
                    for k in range(2):
                        j = 2 * q + k
                        for it in (2, 3):
                            nc.tensor.matmul(yu_of[(h, it)][:, 0:D + 1],
                                             expp[q][:, k * CH + it * 128:k * CH + (it + 1) * 128],
                                             vaug[(h, j)][:, 0:D + 1],
                                             start=(j == 0), stop=(j == NJT - 1))
                # pass-2 normalizes run inside the next head's (or chunk's) bursts
                for it in (2, 3):
                    norm_q.append((b, yTs, ich, h, it, yu_of[(h, it)]))
            while pending:
                emit_proj_task(pending.pop())
            proj_prev = (b, ich, yTs)

        # trailing work for this batch's last i-chunk: pit0/pit1 tasks have
        # ready yT tiles, so they lead off the PE stream while the norm
        # chain for (h1,it2),(h1,it3) drains on DVE in parallel; on the
        # final batch, the staging copies alternate DVE/Act and the last
        # row block DMAs per column block to shorten the end drain
        tail_tasks = proj_tasks_of(*proj_prev)
        for it in range(CH // 128):
            for i, task in enumerate(tail_tasks[4 * it:4 * it + 4]):
                if norm_q and i in (0, 1):
                    run_norm_a(norm_q.pop(0))
                if normb_q and i in (2, 3):
                    run_norm_b(normb_q.pop(0))
                emit_proj_task(task, tail=(b == B - 1),
                               final=(b == B - 1 and it == CH // 128 - 1))
        proj_prev = None

    ctx.close()


def make_nc(B=2, T=2048, C=2048, reps=1, loop=1):
    nc = bacc.Bacc("TRN2", target_bir_lowering=False, debug=False)
    nc.tensor_map = {}

    def dram(name, shape, kind, dt=F32):
        t = nc.dram_tensor(name, shape, dt, kind=kind)
        nc.tensor_map[name] = t
        return t

    CH = 512
    NCH = T // CH
    NCT = C // 128
    dram("xT", [B, NCH, 128, NCT * CH], "ExternalInput", BF16)
    dram("wq", [128, NCT * HLOC * D], "ExternalInput", BF16)
    dram("wk", [128, NCT * HLOC * D], "ExternalInput", BF16)
    dram("wv", [128, NCT * HLOC * D], "ExternalInput", BF16)
    dram("wp", [128, HLOC * T], "ExternalInput", BF16)
    dram("cos_e", [ROT, T], "ExternalInput")
    dram("sin_e", [ROT, T], "ExternalInput")
    dram("perm", [ROT, ROT], "ExternalInput", BF16)
    dram("ident", [128, 128], "ExternalInput", BF16)
    dram("out", [B, T, C], "ExternalOutput", BF16)

    with tile.TileContext(nc) as tc:
        if loop > 1:
            with tc.For_i(0, loop, 1):
                build_core_kernel(nc, tc, B, T, C)
        else:
            for _ in range(reps):
                build_core_kernel(nc, tc, B, T, C)
    nc.compile()
    return nc


def host_inputs(x, W_attn, W_proj, cos, sin):
    """Build the 8 per-core input maps from the full-problem inputs."""
    import ml_dtypes
    BF = ml_dtypes.bfloat16
    B, T, C = x.shape
    CH = 512
    NCH = T // CH
    NCT = C // 128
    # xr[b, ch, p, ct*CH + c] = x[b, ch*CH + c, ct*128 + p]
    xr = np.ascontiguousarray(
        x.reshape(B, NCH, CH, NCT, 128).transpose(0, 1, 4, 3, 2)
        .reshape(B, NCH, 128, NCT * CH)).astype(BF)
    cos_e = np.repeat(np.ascontiguousarray(cos.T), 2, axis=0).astype(np.float32)
    sin_e = np.repeat(np.ascontiguousarray(sin.T), 2, axis=0).astype(np.float32)
    sin_e[0::2, :] *= -1.0
    perm = np.zeros((ROT, ROT), np.float32)
    for i in range(ROT):
        perm[i, i ^ 1] = 1.0
    ident = np.eye(128, dtype=np.float32).astype(BF)
    scale = 1.0 / math.sqrt(D)

    def wlayout(w):
        # [C, HLOC*D] -> [128, NCT * HLOC*D] with ct-major column blocks
        return np.ascontiguousarray(
            w.reshape(NCT, 128, HLOC * D).transpose(1, 0, 2).reshape(128, -1)).astype(BF)

    in_maps = []
    for core in range(N_CORES):
        h0 = core * HLOC
        sl = slice(h0 * D, (h0 + HLOC) * D)
        wp = np.ascontiguousarray(
            W_proj[sl, :].reshape(HLOC, 128, C).transpose(1, 0, 2).reshape(128, -1)).astype(BF)
        in_maps.append({
            "xT": xr,
            "wq": wlayout(W_attn[:, sl] * scale),
            "wk": wlayout(W_attn[:, C + h0 * D: C + (h0 + HLOC) * D]),
            "wv": wlayout(W_attn[:, 2 * C + h0 * D: 2 * C + (h0 + HLOC) * D]),
            "wp": wp,
            "cos_e": cos_e,
            "sin_e": sin_e,
            "perm": perm.astype(BF),
            "ident": ident,
        })
    return in_maps


_NC_CACHE = {}


def build_runner(nc):
    """Build a sharded jitted runner over 8 cores for an arbitrary nc, modeled
    on concourse.bass2jax.run_bass_via_pjrt but with a cached executable."""
    import jax
    from jax.sharding import Mesh, PartitionSpec
    from jax.experimental.shard_map import shard_map
    from concourse import bass2jax

    bass2jax.install_neuronx_cc_hook()

    partition_name = nc.partition_id_tensor.name if nc.partition_id_tensor else None
    in_names, out_names, out_avals, zero_shapes = [], [], [], []
    for alloc in nc.m.functions[0].allocations:
        if not isinstance(alloc, mybir.MemoryLocationSet):
            continue
        name = alloc.memorylocations[0].name
        if alloc.kind == "ExternalInput":
            if name != partition_name:
                in_names.append(name)
        elif alloc.kind == "ExternalOutput":
            out_names.append(name)
            shape = tuple(alloc.tensor_shape)
            dtype = mybir.dt.np(alloc.dtype)
            out_avals.append(jax.core.ShapedArray(shape, dtype))
            zero_shapes.append((shape, dtype))
    n_params = len(in_names)
    n_outs = len(out_names)
    all_names = in_names + out_names
    if partition_name is not None:
        all_names = all_names + [partition_name]

    def _body(*args):
        operands = list(args)
        if partition_name is not None:
            operands.append(bass2jax.partition_id_tensor())
        outs = bass2jax._bass_exec_p.bind(
            *operands,
            out_avals=tuple(out_avals),
            in_names=tuple(all_names),
            out_names=tuple(out_names),
            lowering_input_output_aliases=(),
            sim_require_finite=True,
            sim_require_nnan=True,
            nc=nc,
        )
        return tuple(outs)

    devices = jax.devices()[:N_CORES]
    mesh = Mesh(np.asarray(devices), ("core",))
    in_specs = (PartitionSpec("core"),) * (n_params + n_outs)
    out_specs = (PartitionSpec("core"),) * n_outs
    donate = tuple(range(n_params, n_params + n_outs))
    sharded = jax.jit(
        shard_map(_body, mesh=mesh, in_specs=in_specs, out_specs=out_specs,
                  check_rep=False),
        donate_argnums=donate, keep_unused=True)

    runner = {
        "fn": sharded, "in_names": in_names, "out_names": out_names,
        "zero_shapes": zero_shapes, "n_params": n_params, "mesh": mesh,
    }
    return runner


def _get_runner(reps=1, loop=1):
    key = ("runner", reps, loop)
    if key not in _NC_CACHE:
        _NC_CACHE[key] = build_runner(make_nc(reps=reps, loop=loop))
    return _NC_CACHE[key]


def _concat_inputs(runner, in_maps):
    return [np.concatenate([in_maps[c][name] for c in range(N_CORES)], axis=0)
            for name in runner["in_names"]]


def _make_zeros(runner):
    return [np.zeros((N_CORES * s[0], *s[1:]), dt)
            for (s, dt) in runner["zero_shapes"]]


def kernel(x, W_attn, W_proj, cos, sin):
    x = np.asarray(x, np.float32)
    W_attn = np.asarray(W_attn, np.float32)
    W_proj = np.asarray(W_proj, np.float32)
    cos = np.asarray(cos, np.float32)
    sin = np.asarray(sin, np.float32)

    runner = _get_runner()
    in_maps = host_inputs(x, W_attn, W_proj, cos, sin)
    out_arrs = runner["fn"](*_concat_inputs(runner, in_maps), *_make_zeros(runner))
    B, T, C = x.shape
    parts = np.asarray(out_arrs[0]).astype(np.float32).reshape(N_CORES, B, T, C)
    return parts.sum(axis=0, dtype=np.float32)


def bench(x, W_attn, W_proj, cos, sin, iters=10, reps=1, loop=1):
    """Time device-resident executions; returns (output, per-iter seconds list)."""
    import time
    import jax
    from jax.sharding import NamedSharding, PartitionSpec

    runner = _get_runner(reps=reps, loop=loop)
    in_maps = host_inputs(np.asarray(x, np.float32), np.asarray(W_attn, np.float32),
                          np.asarray(W_proj, np.float32), np.asarray(cos, np.float32),
                          np.asarray(sin, np.float32))
    sharding = NamedSharding(runner["mesh"], PartitionSpec("core"))
    dev_in = [jax.device_put(a, sharding) for a in _concat_inputs(runner, in_maps)]
    zero_sets = [[jax.device_put(z, sharding) for z in _make_zeros(runner)]
                 for _ in range(iters + 1)]
    for z in zero_sets:
        jax.block_until_ready(z)
    jax.block_until_ready(dev_in)

    # warmup (also compiles)
    out = runner["fn"](*dev_in, *zero_sets[0])
    jax.block_until_ready(out)
    times = []
    for i in range(iters):
        t0 = time.perf_counter()
        out = runner["fn"](*dev_in, *zero_sets[i + 1])
        jax.block_until_ready(out)
        times.append(time.perf_counter() - t0)
    B, T, C = np.asarray(x).shape
    parts = np.asarray(out[0]).astype(np.float32).reshape(N_CORES, B, T, C)
    return parts.sum(axis=0, dtype=np.float32), times


if __name__ == "__main__":
    np.random.seed(0)
    B, T, C = 2, 2048, 2048
    x = np.random.randn(B, T, C).astype(np.float32)
    W_attn = (np.random.randn(C, 3 * C) / math.sqrt(C)).astype(np.float32)
    W_proj = (np.random.randn(C, C) / math.sqrt(C)).astype(np.float32)
    half = ROT // 2
    inv = 1.0 / 10000.0 ** (np.arange(half, dtype=np.float32) / half)
    ang = np.outer(np.arange(T, dtype=np.float32), inv)
    out = kernel(x, W_attn, W_proj, np.cos(ang), np.sin(ang))
    print(out.shape, out.dtype)

